# revision 54
# baseline (speedup 1.0000x reference)
"""Trainium2 Bass kernel for nn_MultiHeadAttention (B=4, S=2048, D=1024, H=16).

Sharding: 8 cores = (batch b in 0..3) x (head-half hb in 0..1).
Each core computes, for its batch b and its 8 heads:
  kT = (K[b] @ W_k[hb].T).T        [512, S]   (features on partitions)
  v  =  V[b] @ W_v[hb].T           [S, 512]   (+ ones column per head)
  qT = (Q[b] @ W_q[hb].T).T        [512, S]
  per head-pair pr: scoresT = k x q contracted over d_k -> [sk, sq]
              expT = exp(scoresT/8)  (no max subtraction; scores are O(5))
              PV with ones-row gives unnormalized attn.T and the softmax
              denominator in the same PSUM accumulation (M=65 matmul)
              normalize: denom row -> scatter DMA -> [128,8] reciprocal ->
              gather DMA -> gpsimd broadcast -> two DVE muls
  out_partial = attnT.T @ W_o[:, hb-slice].T      [S, 1024]
Host sums the two partial outputs per batch.

All DMA'd tensors are bf16 (halves HBM traffic); matmuls run bf16 at full
PE rate. Output projection and q-projection chunks are interleaved into
phase 2's scalar-engine (exp) slack as "filler" units at block boundaries.
"""

import sys

sys.path.insert(0, "/opt/trn_rl_repo")

from contextlib import ExitStack

import numpy as np

import concourse.bass as bass  # noqa: F401
import concourse.tile as tile
from concourse import bacc, mybir
from concourse.bass_utils import run_bass_kernel_spmd

F32 = mybir.dt.float32
BF = mybir.dt.bfloat16
EXP = mybir.ActivationFunctionType.Exp

D = 1024  # d_model
HD = 512  # head-dim slice per core (8 heads x 64)
DK = 64
NH = 8  # heads per core
P = 128


def build(S=2048):
    """Build the per-core Bass program (same program for all 8 cores)."""
    nc = bacc.Bacc(
        "TRN2",
        target_bir_lowering=False,
        debug=False,
        enable_asserts=False,
        num_devices=1,
    )

    xqt = nc.dram_tensor("xqt", [D, S], BF, kind="ExternalInput").ap()
    xkt = nc.dram_tensor("xkt", [D, S], BF, kind="ExternalInput").ap()
    xvt = nc.dram_tensor("xvt", [D, S], BF, kind="ExternalInput").ap()
    wqt = nc.dram_tensor("wqt", [D, HD], BF, kind="ExternalInput").ap()
    wkt = nc.dram_tensor("wkt", [D, HD], BF, kind="ExternalInput").ap()
    wvt = nc.dram_tensor("wvt", [D, HD], BF, kind="ExternalInput").ap()
    wot = nc.dram_tensor("wot", [HD, D], BF, kind="ExternalInput").ap()
    out = nc.dram_tensor("out", [S, D], BF, kind="ExternalOutput").ap()

    KO = D // P  # 8 contraction subtiles
    JC = 512  # sq chunk (phase 2)
    n_c = S // JC  # 4
    n_sk = S // P  # 16
    n_st = S // P  # 16 phase-3 s tiles

    with tile.TileContext(nc) as tc, ExitStack() as ctx:
        pers = ctx.enter_context(tc.tile_pool(name="pers", bufs=1))
        wpool = ctx.enter_context(tc.tile_pool(name="wpool", bufs=2))
        xpool = ctx.enter_context(tc.tile_pool(name="xpool", bufs=2))
        epool = ctx.enter_context(tc.tile_pool(name="epool", bufs=3))
        spool = ctx.enter_context(tc.tile_pool(name="spool", bufs=3))
        bpool = ctx.enter_context(tc.tile_pool(name="bpool", bufs=2))
        tpool = ctx.enter_context(tc.tile_pool(name="tpool", bufs=2))
        opool = ctx.enter_context(tc.tile_pool(name="opool", bufs=4))

        # persistent intermediates
        qt = pers.tile([P, 4, S], BF)  # qT: [p, pair, s], feature = pair*128+p
        kt = pers.tile([P, 4, S], BF)
        va = pers.tile([P, n_sk, NH, DK + 1], BF)  # v_aug: [s%128, s//128, h, dv|1]
        at = pers.tile([P, 4, S], BF)  # attnT (normalized)

        nc.vector.memset(va[:, :, :, DK], 1.0)
        ones1 = pers.tile([1, DK], BF)  # PE outer-product broadcast seed
        nc.vector.memset(ones1, 1.0)
        # warm the scalar engine's Exp table during phase 1 so the 1.3us
        # ACT_TABLE_LOAD is off phase-2's critical path
        warm = pers.tile([1, DK], BF)
        nc.scalar.activation(warm, ones1, EXP)

        # ---------------- Phase 1: k, v projections + q(c0) ----------------
        ps1_ctx = ExitStack()
        ps1 = ps1_ctx.enter_context(tc.tile_pool(name="ps1", bufs=2, space="PSUM"))

        def dma_x(dst, src):
            # chunked per k-subtile so the first matmul starts early
            for k in range(KO):
                nc.sync.dma_start(dst[:, k, :], src[k * P : (k + 1) * P, :])

        # k projection: kT [feat, s], stationary w reused across 4 s-chunks
        # weight + x DMAs chunked per k-subtile, interleaved, so the first
        # matmul starts as soon as the first ~0.6MB lands
        wk = wpool.tile([P, KO, HD], BF, tag="w", name="w_k")
        xk = xpool.tile([P, KO, S], BF, tag="x", name="x_k")
        nc.sync.dma_start(wk[:, 0, :], wkt[0:P, :])
        nc.sync.dma_start(xk[:, 0, 0:JC], xkt[0:P, 0:JC])  # first matmul's slice
        nc.sync.dma_start(xk[:, 0, JC:S], xkt[0:P, JC:S])
        for k in range(1, KO):
            nc.sync.dma_start(wk[:, k, :], wkt[k * P : (k + 1) * P, :])
            nc.sync.dma_start(xk[:, k, :], xkt[k * P : (k + 1) * P, :])
        xv = xpool.tile([P, KO, S], BF, tag="x", name="x_v")
        dma_x(xv, xvt)
        for pr in range(4):
            ps = ps1.tile([P, S], F32, tag="ps1", name="ps_k")
            for k in range(KO):
                for cs in range(4):
                    nc.tensor.matmul(
                        ps[:, cs * JC : (cs + 1) * JC],
                        lhsT=wk[:, k, pr * P : (pr + 1) * P],
                        rhs=xk[:, k, cs * JC : (cs + 1) * JC],
                        start=(k == 0),
                        stop=(k == KO - 1),
                        skip_group_check=True,
                    )
            nc.vector.tensor_copy(kt[:, pr, :], ps)

        # v projection: v[s, dv], 4 s-tiles per PSUM tile
        wv = wpool.tile([P, KO, HD], BF, tag="w", name="w_v")
        nc.sync.dma_start(wv, wvt.rearrange("(o p) m -> p o m", p=P))
        for g in range(4):
            ps = ps1.tile([P, 4, HD], F32, tag="ps1", name="ps_v")
            for st in range(4):
                s_tile = 4 * g + st
                for k in range(KO):
                    nc.tensor.matmul(
                        ps[:, st, :],
                        lhsT=xv[:, k, s_tile * P : (s_tile + 1) * P],
                        rhs=wv[:, k, :],
                        start=(k == 0),
                        stop=(k == KO - 1),
                        skip_group_check=True,
                    )
            nc.vector.tensor_copy(
                va[:, 4 * g : 4 * g + 4, :, 0:DK],
                ps.rearrange("p st (h d) -> p st h d", d=DK),
            )

        # q projection weights + input (q chunks c1..c3 become phase-2 filler)
        wq = wpool.tile([P, KO, HD], BF, tag="w", name="w_q")
        nc.sync.dma_start(wq, wqt.rearrange("(o p) m -> p o m", p=P))
        xq = xpool.tile([P, KO, S], BF, tag="x", name="x_q")
        dma_x(xq, xqt)
        wo = wpool.tile([P, 4, D], BF, tag="w", name="w_o")
        nc.sync.dma_start(wo, wot.rearrange("(pr p) n -> p pr n", p=P))

        # q(c0, pr0) in phase-1 PSUM (prefix for the first phase-2 block);
        # q(c0, pr1..3) are deferred to phase-2 boundaries 1..3
        ps = ps1.tile([P, S], F32, tag="ps1", name="ps_q0")
        for k in range(KO):
            nc.tensor.matmul(
                ps[:, 0:JC],
                lhsT=wq[:, k, 0:P],
                rhs=xq[:, k, 0:JC],
                start=(k == 0),
                stop=(k == KO - 1),
                skip_group_check=True,
            )
        nc.vector.tensor_copy(qt[:, 0, 0:JC], ps[:, 0:JC])

        ps1_ctx.close()

        # ---------------- Phase 2 + interleaved fillers ----------------
        ps2_ctx = ExitStack()
        ps_score = ps2_ctx.enter_context(
            tc.tile_pool(name="ps_score", bufs=2, space="PSUM")
        )
        ps_out = ps2_ctx.enter_context(tc.tile_pool(name="ps_out", bufs=2, space="PSUM"))

        def q_unit(pr, c):
            """Project q chunk c for head-pair pr into qt (8 matmuls)."""

            def emit():
                ps = ps_out.tile([P, 2 * JC], F32, tag="po", name="po")
                for k in range(KO):
                    nc.tensor.matmul(
                        ps[:, 0:JC],
                        lhsT=wq[:, k, pr * P : (pr + 1) * P],
                        rhs=xq[:, k, c * JC : (c + 1) * JC],
                        start=(k == 0),
                        stop=(k == KO - 1),
                        skip_group_check=True,
                    )
                nc.vector.tensor_copy(qt[:, pr, c * JC : (c + 1) * JC], ps[:, 0:JC])

            return emit

        def o_unit(st):
            """Output-projection s-tile st (8 matmuls + copy + DMA out)."""

            def emit():
                ps = ps_out.tile([P, 2 * JC], F32, tag="po", name="po")
                for pr in range(4):
                    lhs = at[:, pr, st * P : (st + 1) * P]
                    nc.tensor.matmul(
                        ps[:, 0:512],
                        lhsT=lhs,
                        rhs=wo[:, pr, 0:512],
                        start=(pr == 0),
                        stop=(pr == 3),
                        skip_group_check=True,
                    )
                    nc.tensor.matmul(
                        ps[:, 512:1024],
                        lhsT=lhs,
                        rhs=wo[:, pr, 512:1024],
                        start=(pr == 0),
                        stop=(pr == 3),
                        skip_group_check=True,
                    )
                ob = opool.tile([P, D], BF, tag="ob", name="ob")
                nc.vector.tensor_copy(ob, ps)
                nc.sync.dma_start(out[st * P : (st + 1) * P, :], ob)

            return emit

        # filler schedule: boundary index b (0..15) = before block b; 16 = end.
        # block b = (c = b // 4, pr = b % 4).
        fillers = {b: [] for b in range(17)}
        for pr in range(1, 4):
            fillers[pr].append(q_unit(pr, 0))  # just-in-time for blocks 1..3
        for pr in range(4):
            fillers[1 + pr].append(q_unit(pr, 1))  # during c0 blocks
            fillers[5 + pr].append(q_unit(pr, 2))  # during c1 blocks
            fillers[9 + pr].append(q_unit(pr, 3))  # during c2 blocks
        for i in range(4):  # phase-3 tiles for chunk c, after chunk c done
            fillers[5 + i].append(o_unit(0 * 4 + i))
            fillers[9 + i].append(o_unit(1 * 4 + i))
        for i in range(3):
            fillers[13 + i].append(o_unit(8 + i))
        fillers[15].append(o_unit(11))  # keep the tail free for c3 tiles
        # chunk-c3 output tiles are handled by the restructured tail below

        for c in range(n_c):
            cs = slice(c * JC, (c + 1) * JC)
            for pr in range(4):
                block = c * 4 + pr
                for f in fillers[block]:
                    f()
                last = block == 15
                if last:
                    # PSUM landing zone for the final block's PE-broadcast
                    # reciprocal (reserved before po so the ring stays sane)
                    bc_ps = ps_out.tile([P, 2 * JC], F32, tag="po", name="bc_ps")
                qa, qb = qt[0:DK, pr, cs], qt[DK:P, pr, cs]
                ka, kb = kt[0:DK, pr, :], kt[DK:P, pr, :]
                ha, hb = 2 * pr, 2 * pr + 1
                po = ps_out.tile([P, 2 * JC], F32, tag="po", name="po")
                for sk in range(n_sk):
                    ks = slice(sk * P, (sk + 1) * P)
                    pss = ps_score.tile([P, 2 * JC], F32, tag="pss", name="pss")
                    nc.tensor.matmul(pss[:, 0:JC], lhsT=ka[:, ks], rhs=qa,
                                     start=True, stop=True)
                    nc.tensor.matmul(pss[:, JC : 2 * JC], lhsT=kb[:, ks], rhs=qb,
                                     start=True, stop=True)
                    ex = epool.tile([P, 2 * JC], BF, tag="ex", name="ex")
                    nc.scalar.activation(ex, pss, EXP, scale=0.125)
                    nc.tensor.matmul(po[0:DK + 1, 0:JC], lhsT=va[:, sk, ha, :],
                                     rhs=ex[:, 0:JC],
                                     start=(sk == 0), stop=(sk == n_sk - 1))
                    nc.tensor.matmul(po[0:DK + 1, JC : 2 * JC], lhsT=va[:, sk, hb, :],
                                     rhs=ex[:, JC : 2 * JC],
                                     start=(sk == 0), stop=(sk == n_sk - 1))
                # normalization: batched reciprocal across partitions
                den = spool.tile([DK + 1, 2 * JC], F32, tag="den", name="den")
                if last:
                    # scalar engine is drained by now; skip the DVE queue
                    nc.scalar.copy(den[DK : DK + 1, :], po[DK : DK + 1, :])
                else:
                    nc.vector.tensor_copy(den[DK : DK + 1, :], po[DK : DK + 1, :])
                d8 = spool.tile([P, 8], F32, tag="d8", name="d8")
                nc.sync.dma_start(d8, den[DK : DK + 1, :])
                if last:
                    # bf16 reciprocal -> PE outer-product broadcast (PE is
                    # idle here; avoids the ~3us gpsimd dispatch latency)
                    r8b = spool.tile([P, 8], BF, tag="r8", name="r8b")
                    with nc.allow_low_precision("bf16 recip: 0.4% on 1/16 blocks"):
                        nc.vector.reciprocal(r8b, d8)
                    r0b = spool.tile([1, 2 * JC], BF, tag="r0", name="r0b")
                    nc.sync.dma_start(r0b, r8b)
                    for half in range(2):
                        nc.tensor.matmul(
                            bc_ps[0:DK, half * JC : (half + 1) * JC],
                            lhsT=ones1,
                            rhs=r0b[:, half * JC : (half + 1) * JC],
                            start=True,
                            stop=True,
                        )
                    # tensor_tensor cannot take two PSUM operands; bounce
                    bc = bpool.tile([DK, 2 * JC], F32, tag="bc", name="bc")
                    nc.vector.tensor_copy(bc, bc_ps[0:DK, :])
                else:
                    r8 = spool.tile([P, 8], F32, tag="r8", name="r8")
                    nc.vector.reciprocal(r8, d8)
                    r0 = spool.tile([1, 2 * JC], F32, tag="r0", name="r0")
                    nc.sync.dma_start(r0, r8)
                    bc = bpool.tile([DK, 2 * JC], F32, tag="bc", name="bc")
                    nc.gpsimd.partition_broadcast(bc, r0, channels=DK)
                nc.vector.tensor_mul(at[0:DK, pr, cs], po[0:DK, 0:JC], bc[0:DK, 0:JC])
                tt = tpool.tile([DK, JC], BF, tag="tt", name="tt")
                nc.vector.tensor_mul(tt, po[0:DK, JC : 2 * JC], bc[0:DK, JC : 2 * JC])
                nc.sync.dma_start(at[DK:P, pr, cs], tt)

        for f in fillers[16]:
            f()

        # ---- restructured tail for chunk c3's output tiles ----
        # st12/st13: pr0..2 contributions run on the PE while the last
        # block's normalization chain (DVE/DMA/gpsimd) is still in flight;
        # only the 2-matmul pr3 parts wait for it. st14/st15 follow whole.
        pre = {}
        for st in (12, 13):
            ps_t = ps_score.tile([P, 2 * JC], F32, tag="pss", name="pss_o3")
            pre[st] = ps_t
            for pr in range(3):
                for half in range(2):
                    nc.tensor.matmul(
                        ps_t[:, half * 512 : (half + 1) * 512],
                        lhsT=at[:, pr, st * P : (st + 1) * P],
                        rhs=wo[:, pr, half * 512 : (half + 1) * 512],
                        start=(pr == 0),
                        stop=False,
                        skip_group_check=True,
                    )
        for st in (12, 13):
            ps_t = pre[st]
            for half in range(2):
                nc.tensor.matmul(
                    ps_t[:, half * 512 : (half + 1) * 512],
                    lhsT=at[:, 3, st * P : (st + 1) * P],
                    rhs=wo[:, 3, half * 512 : (half + 1) * 512],
                    start=False,
                    stop=True,
                    skip_group_check=True,
                )
            ob = opool.tile([P, D], BF, tag="ob", name="ob")
            nc.vector.tensor_copy(ob, ps_t)
            nc.sync.dma_start(out[st * P : (st + 1) * P, :], ob)
        o_unit(14)()
        o_unit(15)()
        ps2_ctx.close()

    nc.compile()
    return nc


_nc_cache = {}


def _get_nc(S=2048):
    if S not in _nc_cache:
        _nc_cache[S] = build(S)
    return _nc_cache[S]


def _bf16(x):
    import ml_dtypes

    return np.ascontiguousarray(x).astype(ml_dtypes.bfloat16)


def make_in_maps(Q, K, V, W_q, W_k, W_v, W_o):
    Q, K, V = (np.asarray(t, dtype=np.float32) for t in (Q, K, V))
    W_q, W_k, W_v, W_o = (np.asarray(t, dtype=np.float32) for t in (W_q, W_k, W_v, W_o))
    in_maps = []
    for c in range(8):
        b, hb = c // 2, c % 2
        sl = slice(hb * HD, (hb + 1) * HD)
        in_maps.append(
            {
                "xqt": _bf16(Q[b].T),
                "xkt": _bf16(K[b].T),
                "xvt": _bf16(V[b].T),
                "wqt": _bf16(W_q[sl, :].T),
                "wkt": _bf16(W_k[sl, :].T),
                "wvt": _bf16(W_v[sl, :].T),
                "wot": _bf16(W_o[:, sl].T),
            }
        )
    return in_maps


def kernel(Q, K, V, W_q, W_k, W_v, W_o):
    nc = _get_nc(2048)
    in_maps = make_in_maps(Q, K, V, W_q, W_k, W_v, W_o)
    res = run_bass_kernel_spmd(nc, in_maps, core_ids=list(range(8)))
    outs = [res.results[c]["out"].astype(np.float32) for c in range(8)]
    full = np.stack([outs[2 * b] + outs[2 * b + 1] for b in range(4)], axis=0)
    return full.astype(np.float32)


# revision 57
# speedup vs baseline: 1.0031x; 1.0031x over previous
"""Trainium2 Bass kernel for nn_MultiHeadAttention (B=4, S=2048, D=1024, H=16).

Sharding: 8 cores = (batch b in 0..3) x (head-half hb in 0..1).
Each core computes, for its batch b and its 8 heads:
  kT = (K[b] @ W_k[hb].T).T        [512, S]   (features on partitions)
  v  =  V[b] @ W_v[hb].T           [S, 512]   (+ ones column per head)
  qT = (Q[b] @ W_q[hb].T).T        [512, S]
  per head-pair pr: scoresT = k x q contracted over d_k -> [sk, sq]
              expT = exp(scoresT/8)  (no max subtraction; scores are O(5))
              PV with ones-row gives unnormalized attn.T and the softmax
              denominator in the same PSUM accumulation (M=65 matmul)
              normalize: denom row -> scatter DMA -> [128,8] reciprocal ->
              gather DMA -> gpsimd broadcast -> two DVE muls
  out_partial = attnT.T @ W_o[:, hb-slice].T      [S, 1024]
Host sums the two partial outputs per batch.

All DMA'd tensors are bf16 (halves HBM traffic); matmuls run bf16 at full
PE rate. Output projection and q-projection chunks are interleaved into
phase 2's scalar-engine (exp) slack as "filler" units at block boundaries.
"""

import sys

sys.path.insert(0, "/opt/trn_rl_repo")

from contextlib import ExitStack

import numpy as np

import concourse.bass as bass  # noqa: F401
import concourse.tile as tile
from concourse import bacc, mybir
from concourse.bass_utils import run_bass_kernel_spmd

F32 = mybir.dt.float32
BF = mybir.dt.bfloat16
EXP = mybir.ActivationFunctionType.Exp

D = 1024  # d_model
HD = 512  # head-dim slice per core (8 heads x 64)
DK = 64
NH = 8  # heads per core
P = 128


def build(S=2048):
    """Build the per-core Bass program (same program for all 8 cores)."""
    nc = bacc.Bacc(
        "TRN2",
        target_bir_lowering=False,
        debug=False,
        enable_asserts=False,
        num_devices=1,
    )

    xqt = nc.dram_tensor("xqt", [D, S], BF, kind="ExternalInput").ap()
    xkt = nc.dram_tensor("xkt", [D, S], BF, kind="ExternalInput").ap()
    xvt = nc.dram_tensor("xvt", [D, S], BF, kind="ExternalInput").ap()
    wqt = nc.dram_tensor("wqt", [D, HD], BF, kind="ExternalInput").ap()
    wkt = nc.dram_tensor("wkt", [D, HD], BF, kind="ExternalInput").ap()
    wvt = nc.dram_tensor("wvt", [D, HD], BF, kind="ExternalInput").ap()
    wot = nc.dram_tensor("wot", [HD, D], BF, kind="ExternalInput").ap()
    out = nc.dram_tensor("out", [S, D], BF, kind="ExternalOutput").ap()

    KO = D // P  # 8 contraction subtiles
    JC = 512  # sq chunk (phase 2)
    n_c = S // JC  # 4
    n_sk = S // P  # 16
    n_st = S // P  # 16 phase-3 s tiles

    with tile.TileContext(nc) as tc, ExitStack() as ctx:
        pers = ctx.enter_context(tc.tile_pool(name="pers", bufs=1))
        wpool = ctx.enter_context(tc.tile_pool(name="wpool", bufs=2))
        xpool = ctx.enter_context(tc.tile_pool(name="xpool", bufs=2))
        epool = ctx.enter_context(tc.tile_pool(name="epool", bufs=3))
        spool = ctx.enter_context(tc.tile_pool(name="spool", bufs=3))
        bpool = ctx.enter_context(tc.tile_pool(name="bpool", bufs=2))
        tpool = ctx.enter_context(tc.tile_pool(name="tpool", bufs=2))
        opool = ctx.enter_context(tc.tile_pool(name="opool", bufs=4))

        # persistent intermediates
        qt = pers.tile([P, 4, S], BF)  # qT: [p, pair, s], feature = pair*128+p
        kt = pers.tile([P, 4, S], BF)
        va = pers.tile([P, n_sk, NH, DK + 1], BF)  # v_aug: [s%128, s//128, h, dv|1]
        at = pers.tile([P, 4, S], BF)  # attnT (normalized)

        nc.vector.memset(va[:, :, :, DK], 1.0)
        ones1 = pers.tile([1, DK], BF)  # PE outer-product broadcast seed
        nc.vector.memset(ones1, 1.0)
        # warm the scalar engine's Exp table during phase 1 so the 1.3us
        # ACT_TABLE_LOAD is off phase-2's critical path
        warm = pers.tile([1, DK], BF)
        nc.scalar.activation(warm, ones1, EXP)

        # ---------------- Phase 1: k, v projections + q(c0) ----------------
        ps1_ctx = ExitStack()
        ps1 = ps1_ctx.enter_context(tc.tile_pool(name="ps1", bufs=2, space="PSUM"))

        def dma_x(dst, src):
            # chunked per k-subtile so the first matmul starts early
            for k in range(KO):
                nc.sync.dma_start(dst[:, k, :], src[k * P : (k + 1) * P, :])

        # k projection: kT [feat, s], stationary w reused across 4 s-chunks
        # weight + x DMAs chunked per k-subtile, interleaved, so the first
        # matmul starts as soon as the first ~0.6MB lands
        wk = wpool.tile([P, KO, HD], BF, tag="w", name="w_k")
        xk = xpool.tile([P, KO, S], BF, tag="x", name="x_k")
        nc.sync.dma_start(wk[:, 0, :], wkt[0:P, :])
        for cs in range(4):  # first k-subtile in score-chunk pieces so the
            nc.sync.dma_start(  # first matmuls start incrementally
                xk[:, 0, cs * JC : (cs + 1) * JC], xkt[0:P, cs * JC : (cs + 1) * JC]
            )
        for k in range(1, KO):
            nc.sync.dma_start(wk[:, k, :], wkt[k * P : (k + 1) * P, :])
            nc.sync.dma_start(xk[:, k, :], xkt[k * P : (k + 1) * P, :])
        # wv interleaved with xv per k-subtile: v-proj's first matmul needs
        # wv[k0], which must not queue behind all 4MB of xv
        wv = wpool.tile([P, KO, HD], BF, tag="w", name="w_v")
        xv = xpool.tile([P, KO, S], BF, tag="x", name="x_v")
        for k in range(KO):
            nc.sync.dma_start(wv[:, k, :], wvt[k * P : (k + 1) * P, :])
            nc.sync.dma_start(xv[:, k, :], xvt[k * P : (k + 1) * P, :])
        for pr in range(4):
            ps = ps1.tile([P, S], F32, tag="ps1", name="ps_k")
            for k in range(KO):
                for cs in range(4):
                    nc.tensor.matmul(
                        ps[:, cs * JC : (cs + 1) * JC],
                        lhsT=wk[:, k, pr * P : (pr + 1) * P],
                        rhs=xk[:, k, cs * JC : (cs + 1) * JC],
                        start=(k == 0),
                        stop=(k == KO - 1),
                        skip_group_check=True,
                    )
            nc.vector.tensor_copy(kt[:, pr, :], ps)

        # v projection: v[s, dv], 4 s-tiles per PSUM tile
        for g in range(4):
            ps = ps1.tile([P, 4, HD], F32, tag="ps1", name="ps_v")
            for st in range(4):
                s_tile = 4 * g + st
                for k in range(KO):
                    nc.tensor.matmul(
                        ps[:, st, :],
                        lhsT=xv[:, k, s_tile * P : (s_tile + 1) * P],
                        rhs=wv[:, k, :],
                        start=(k == 0),
                        stop=(k == KO - 1),
                        skip_group_check=True,
                    )
            nc.vector.tensor_copy(
                va[:, 4 * g : 4 * g + 4, :, 0:DK],
                ps.rearrange("p st (h d) -> p st h d", d=DK),
            )

        # q projection weights + input (q chunks c1..c3 become phase-2 filler)
        wq = wpool.tile([P, KO, HD], BF, tag="w", name="w_q")
        nc.sync.dma_start(wq, wqt.rearrange("(o p) m -> p o m", p=P))
        xq = xpool.tile([P, KO, S], BF, tag="x", name="x_q")
        dma_x(xq, xqt)
        wo = wpool.tile([P, 4, D], BF, tag="w", name="w_o")
        nc.sync.dma_start(wo, wot.rearrange("(pr p) n -> p pr n", p=P))

        # q(c0, pr0) in phase-1 PSUM (prefix for the first phase-2 block);
        # q(c0, pr1..3) are deferred to phase-2 boundaries 1..3
        ps = ps1.tile([P, S], F32, tag="ps1", name="ps_q0")
        for k in range(KO):
            nc.tensor.matmul(
                ps[:, 0:JC],
                lhsT=wq[:, k, 0:P],
                rhs=xq[:, k, 0:JC],
                start=(k == 0),
                stop=(k == KO - 1),
                skip_group_check=True,
            )
        nc.vector.tensor_copy(qt[:, 0, 0:JC], ps[:, 0:JC])

        ps1_ctx.close()

        # ---------------- Phase 2 + interleaved fillers ----------------
        ps2_ctx = ExitStack()
        ps_score = ps2_ctx.enter_context(
            tc.tile_pool(name="ps_score", bufs=2, space="PSUM")
        )
        ps_out = ps2_ctx.enter_context(tc.tile_pool(name="ps_out", bufs=2, space="PSUM"))

        def q_unit(pr, c):
            """Project q chunk c for head-pair pr into qt (8 matmuls)."""

            def emit():
                ps = ps_out.tile([P, 2 * JC], F32, tag="po", name="po")
                for k in range(KO):
                    nc.tensor.matmul(
                        ps[:, 0:JC],
                        lhsT=wq[:, k, pr * P : (pr + 1) * P],
                        rhs=xq[:, k, c * JC : (c + 1) * JC],
                        start=(k == 0),
                        stop=(k == KO - 1),
                        skip_group_check=True,
                    )
                nc.vector.tensor_copy(qt[:, pr, c * JC : (c + 1) * JC], ps[:, 0:JC])

            return emit

        def o_unit(st):
            """Output-projection s-tile st (8 matmuls + copy + DMA out)."""

            def emit():
                ps = ps_out.tile([P, 2 * JC], F32, tag="po", name="po")
                for pr in range(4):
                    lhs = at[:, pr, st * P : (st + 1) * P]
                    nc.tensor.matmul(
                        ps[:, 0:512],
                        lhsT=lhs,
                        rhs=wo[:, pr, 0:512],
                        start=(pr == 0),
                        stop=(pr == 3),
                        skip_group_check=True,
                    )
                    nc.tensor.matmul(
                        ps[:, 512:1024],
                        lhsT=lhs,
                        rhs=wo[:, pr, 512:1024],
                        start=(pr == 0),
                        stop=(pr == 3),
                        skip_group_check=True,
                    )
                ob = opool.tile([P, D], BF, tag="ob", name="ob")
                nc.vector.tensor_copy(ob, ps)
                nc.sync.dma_start(out[st * P : (st + 1) * P, :], ob)

            return emit

        # filler schedule: boundary index b (0..15) = before block b; 16 = end.
        # block b = (c = b // 4, pr = b % 4).
        fillers = {b: [] for b in range(17)}
        for pr in range(1, 4):
            fillers[pr].append(q_unit(pr, 0))  # just-in-time for blocks 1..3
        for pr in range(4):
            fillers[1 + pr].append(q_unit(pr, 1))  # during c0 blocks
            fillers[5 + pr].append(q_unit(pr, 2))  # during c1 blocks
            fillers[9 + pr].append(q_unit(pr, 3))  # during c2 blocks
        for i in range(4):  # phase-3 tiles for chunk c, after chunk c done
            fillers[5 + i].append(o_unit(0 * 4 + i))
            fillers[9 + i].append(o_unit(1 * 4 + i))
        for i in range(3):
            fillers[13 + i].append(o_unit(8 + i))
        fillers[15].append(o_unit(11))  # keep the tail free for c3 tiles
        # chunk-c3 output tiles are handled by the restructured tail below

        for c in range(n_c):
            cs = slice(c * JC, (c + 1) * JC)
            for pr in range(4):
                block = c * 4 + pr
                for f in fillers[block]:
                    f()
                last = block == 15
                if last:
                    # PSUM landing zone for the final block's PE-broadcast
                    # reciprocal (reserved before po so the ring stays sane)
                    bc_ps = ps_out.tile([P, 2 * JC], F32, tag="po", name="bc_ps")
                qa, qb = qt[0:DK, pr, cs], qt[DK:P, pr, cs]
                ka, kb = kt[0:DK, pr, :], kt[DK:P, pr, :]
                ha, hb = 2 * pr, 2 * pr + 1
                po = ps_out.tile([P, 2 * JC], F32, tag="po", name="po")
                for sk in range(n_sk):
                    ks = slice(sk * P, (sk + 1) * P)
                    pss = ps_score.tile([P, 2 * JC], F32, tag="pss", name="pss")
                    nc.tensor.matmul(pss[:, 0:JC], lhsT=ka[:, ks], rhs=qa,
                                     start=True, stop=True)
                    nc.tensor.matmul(pss[:, JC : 2 * JC], lhsT=kb[:, ks], rhs=qb,
                                     start=True, stop=True)
                    ex = epool.tile([P, 2 * JC], BF, tag="ex", name="ex")
                    nc.scalar.activation(ex, pss, EXP, scale=0.125)
                    nc.tensor.matmul(po[0:DK + 1, 0:JC], lhsT=va[:, sk, ha, :],
                                     rhs=ex[:, 0:JC],
                                     start=(sk == 0), stop=(sk == n_sk - 1))
                    nc.tensor.matmul(po[0:DK + 1, JC : 2 * JC], lhsT=va[:, sk, hb, :],
                                     rhs=ex[:, JC : 2 * JC],
                                     start=(sk == 0), stop=(sk == n_sk - 1))
                # normalization: batched reciprocal across partitions
                den = spool.tile([DK + 1, 2 * JC], F32, tag="den", name="den")
                if last:
                    # scalar engine is drained by now; skip the DVE queue
                    nc.scalar.copy(den[DK : DK + 1, :], po[DK : DK + 1, :])
                else:
                    nc.vector.tensor_copy(den[DK : DK + 1, :], po[DK : DK + 1, :])
                d8 = spool.tile([P, 8], F32, tag="d8", name="d8")
                nc.sync.dma_start(d8, den[DK : DK + 1, :])
                if last:
                    # bf16 reciprocal -> PE outer-product broadcast (PE is
                    # idle here; avoids the ~3us gpsimd dispatch latency)
                    r8b = spool.tile([P, 8], BF, tag="r8", name="r8b")
                    with nc.allow_low_precision("bf16 recip: 0.4% on 1/16 blocks"):
                        nc.vector.reciprocal(r8b, d8)
                    r0b = spool.tile([1, 2 * JC], BF, tag="r0", name="r0b")
                    nc.sync.dma_start(r0b, r8b)
                    for half in range(2):
                        nc.tensor.matmul(
                            bc_ps[0:DK, half * JC : (half + 1) * JC],
                            lhsT=ones1,
                            rhs=r0b[:, half * JC : (half + 1) * JC],
                            start=True,
                            stop=True,
                        )
                    # tensor_tensor cannot take two PSUM operands; bounce
                    bc = bpool.tile([DK, 2 * JC], F32, tag="bc", name="bc")
                    nc.vector.tensor_copy(bc, bc_ps[0:DK, :])
                else:
                    r8 = spool.tile([P, 8], F32, tag="r8", name="r8")
                    nc.vector.reciprocal(r8, d8)
                    r0 = spool.tile([1, 2 * JC], F32, tag="r0", name="r0")
                    nc.sync.dma_start(r0, r8)
                    bc = bpool.tile([DK, 2 * JC], F32, tag="bc", name="bc")
                    nc.gpsimd.partition_broadcast(bc, r0, channels=DK)
                nc.vector.tensor_mul(at[0:DK, pr, cs], po[0:DK, 0:JC], bc[0:DK, 0:JC])
                tt = tpool.tile([DK, JC], BF, tag="tt", name="tt")
                nc.vector.tensor_mul(tt, po[0:DK, JC : 2 * JC], bc[0:DK, JC : 2 * JC])
                nc.sync.dma_start(at[DK:P, pr, cs], tt)

        for f in fillers[16]:
            f()

        # ---- restructured tail for chunk c3's output tiles ----
        # st12/st13: pr0..2 contributions run on the PE while the last
        # block's normalization chain (DVE/DMA/gpsimd) is still in flight;
        # only the 2-matmul pr3 parts wait for it. st14/st15 follow whole.
        pre = {}
        for st in (12, 13):
            ps_t = ps_score.tile([P, 2 * JC], F32, tag="pss", name="pss_o3")
            pre[st] = ps_t
            for pr in range(3):
                for half in range(2):
                    nc.tensor.matmul(
                        ps_t[:, half * 512 : (half + 1) * 512],
                        lhsT=at[:, pr, st * P : (st + 1) * P],
                        rhs=wo[:, pr, half * 512 : (half + 1) * 512],
                        start=(pr == 0),
                        stop=False,
                        skip_group_check=True,
                    )
        for st in (12, 13):
            ps_t = pre[st]
            for half in range(2):
                nc.tensor.matmul(
                    ps_t[:, half * 512 : (half + 1) * 512],
                    lhsT=at[:, 3, st * P : (st + 1) * P],
                    rhs=wo[:, 3, half * 512 : (half + 1) * 512],
                    start=False,
                    stop=True,
                    skip_group_check=True,
                )
            ob = opool.tile([P, D], BF, tag="ob", name="ob")
            nc.vector.tensor_copy(ob, ps_t)
            nc.sync.dma_start(out[st * P : (st + 1) * P, :], ob)
        o_unit(14)()
        o_unit(15)()
        ps2_ctx.close()

    nc.compile()
    return nc


_nc_cache = {}


def _get_nc(S=2048):
    if S not in _nc_cache:
        _nc_cache[S] = build(S)
    return _nc_cache[S]


def _bf16(x):
    import ml_dtypes

    return np.ascontiguousarray(x).astype(ml_dtypes.bfloat16)


def make_in_maps(Q, K, V, W_q, W_k, W_v, W_o):
    Q, K, V = (np.asarray(t, dtype=np.float32) for t in (Q, K, V))
    W_q, W_k, W_v, W_o = (np.asarray(t, dtype=np.float32) for t in (W_q, W_k, W_v, W_o))
    in_maps = []
    for c in range(8):
        b, hb = c // 2, c % 2
        sl = slice(hb * HD, (hb + 1) * HD)
        in_maps.append(
            {
                "xqt": _bf16(Q[b].T),
                "xkt": _bf16(K[b].T),
                "xvt": _bf16(V[b].T),
                "wqt": _bf16(W_q[sl, :].T),
                "wkt": _bf16(W_k[sl, :].T),
                "wvt": _bf16(W_v[sl, :].T),
                "wot": _bf16(W_o[:, sl].T),
            }
        )
    return in_maps


def kernel(Q, K, V, W_q, W_k, W_v, W_o):
    nc = _get_nc(2048)
    in_maps = make_in_maps(Q, K, V, W_q, W_k, W_v, W_o)
    res = run_bass_kernel_spmd(nc, in_maps, core_ids=list(range(8)))
    outs = [res.results[c]["out"].astype(np.float32) for c in range(8)]
    full = np.stack([outs[2 * b] + outs[2 * b + 1] for b in range(4)], axis=0)
    return full.astype(np.float32)
